# revision 18
# baseline (speedup 1.0000x reference)
"""Block-local self-attention (BLOCK=128, 3-block sliding window + global token 0)
for Trainium2, sharded over 8 NeuronCores by (batch*head).

Full shapes: q/k/v (2, 16, 4096, 64) fp32, mask (2, 1, 1, 4096) fp32 (zeros).
Core c handles 4 consecutive (n*16+h) heads, as 2 "head pairs".

Host prepares compute-ready, DMA-friendly layouts (big contiguous descriptors):
  - qt/kt: (pair, 128, T) bf16: rows 0-63 = head A's d, 64-127 = head B's d.
  - vt: (head, 128, NB, 65) bf16: partition = token%128, free = (block, d);
    col 64 = ones (softmax denominator trick).
  - pgq: (head, 128, NB) bf16: exp(scale*K0.Q) in q-partition layout,
    zeroed for query blocks 0,1 (global-token probability, host-computed).
  - v0g: (head, 128, 4, 64) bf16: V[token 0] replicated across partitions
    and the 4 query blocks of a window (for the DVE outer-product add).
  - o: (head, 128, NB, D) fp32 staging layout; host un-permutes after.

Device kernel per (pair, window of 512 queries), software-pipelined:
  - scores for BOTH heads in one (128, 3072) PSUM tile, S^T (key-partition)
    layout: per key block j, two row-tiled matmuls (head A on array rows
    0-63, head B on rows 64-127) run concurrently in the PE array.
  - one exp on ScalarE (scale folded) -> P^T bf16 (128, 3072).
  - PV in q-partition layout: for each (query block c, key block j),
    matmul(lhsT=P_j[:, c cols], rhs=vt_j) accumulates ctx (128, 4, 65)
    PSUM; col 64 = denominator via the ones column. N=65 per matmul, so
    PV streams 780 cols/head/window instead of 1536.
  - normalize on DVE directly from PSUM: denom += pgq, reciprocal,
    ctx += pgq (x) V0 (global-token term), multiply -> fp32 out staging.
Query token 0 (attends the full sequence) is host-computed and patched in.
"""

import math

import numpy as np
import ml_dtypes

N_, H, T, D = 2, 16, 4096, 64
B = 128
NB = T // B            # 32 key/query blocks
HPC = 4                # heads per core
NCORES = 8
WQ = 512               # queries per window
NWIN = T // WQ         # 8 windows per head
SCALE = 1.0 / math.sqrt(D)
BANK = 512             # fp32 elements per PSUM bank (per partition)
VW = D + 1             # vt free width: 64 d + 1 ones


def _window_pieces(w):
    """Pieces for window w: (j, qb_lo, qb_hi, N) with q blocks absolute."""
    qb0, qb1 = 4 * w, 4 * w + 3
    out = []
    for j in range(max(0, qb0 - 1), min(NB - 1, qb1 + 1) + 1):
        qlo = max(qb0, j - 1)
        qhi = min(qb1, j + 1)
        out.append((j, qlo, qhi, (qhi - qlo + 1) * B))
    return out


def _pack_offsets(sizes):
    """Pack piece sizes contiguously from 0 s.t. no piece crosses a 512-elem
    PSUM bank boundary. Returns list of offsets (same order as sizes)."""
    import itertools

    n = len(sizes)
    for perm in itertools.permutations(range(n)):
        off = 0
        offs = [0] * n
        ok = True
        for i in perm:
            sz = sizes[i]
            if off // BANK != (off + sz - 1) // BANK:
                ok = False
                break
            offs[i] = off
            off += sz
        if ok:
            return offs
    raise ValueError(f"cannot pack {sizes}")


_NC_CACHE = {}


def _build_nc():
    if "nc" in _NC_CACHE:
        return _NC_CACHE["nc"]

    import concourse.bacc as bacc
    import concourse.mybir as mybir
    import concourse.tile as tile

    dt = mybir.dt
    F32, BF16 = dt.float32, dt.bfloat16
    HB = 3 * BANK  # per-head columns in the scores tile

    nc = bacc.Bacc("TRN2", target_bir_lowering=False, debug=False)
    qt_d = nc.dram_tensor("qt", [2, 128, T], BF16, kind="ExternalInput")
    kt_d = nc.dram_tensor("kt", [2, 128, T], BF16, kind="ExternalInput")
    vt_d = nc.dram_tensor("vt", [HPC, 128, NB, VW], BF16, kind="ExternalInput")
    pgq_d = nc.dram_tensor("pgq", [HPC, 128, NB], BF16, kind="ExternalInput")
    v0g_d = nc.dram_tensor("v0g", [HPC, 128, 4, D], BF16, kind="ExternalInput")
    o_d = nc.dram_tensor("o", [HPC, 128, NB, D], F32, kind="ExternalOutput")

    with tile.TileContext(nc) as tc:
        with (
            tc.tile_pool(name="singles", bufs=1) as singles,
            tc.tile_pool(name="pp", bufs=2) as pp,
            tc.tile_pool(name="gp", bufs=2) as gp,
            tc.tile_pool(name="up", bufs=2) as up,
            tc.tile_pool(name="rtp", bufs=2) as rtp,
            tc.tile_pool(name="outp", bufs=1) as outp,
            tc.tile_pool(name="spsum", bufs=1, space="PSUM") as spsum,
            tc.tile_pool(name="cpsum", bufs=2, space="PSUM") as cpsum,
        ):
            # Warm the ScalarE exp table during the DMA ramp.
            warm_in = singles.tile([1, 8], F32, tag="warm_in")
            nc.vector.memset(warm_in[:, :], 0.0)
            warm_out = singles.tile([1, 8], BF16, tag="warm_out")
            nc.scalar.activation(
                out=warm_out[:, :],
                in_=warm_in[:, :],
                func=mybir.ActivationFunctionType.Exp,
            )

            # Input loads: plain SWDGE (gpsimd) big contiguous transfers,
            # ordered so pair-0 compute starts ASAP. qt/kt split in two
            # chunks so the first window's blocks arrive early.
            qt_pair, kt_pair = [None] * 2, [None] * 2
            vt, pgq, v0g = [None] * HPC, [None] * HPC, [None] * HPC
            SPL = 6 * B  # first chunk: blocks 0-5 (covers window 0)
            for pair in range(2):
                hA, hB = 2 * pair, 2 * pair + 1
                kt = singles.tile([128, T], BF16, tag=f"kt{pair}")
                qt = singles.tile([128, T], BF16, tag=f"qt{pair}")
                if pair == 0:
                    nc.sync.dma_start(out=kt[:, 0:SPL], in_=kt_d.ap()[0, :, 0:SPL])
                    nc.sync.dma_start(out=qt[:, 0:SPL], in_=qt_d.ap()[0, :, 0:SPL])
                else:
                    nc.gpsimd.dma_start(out=kt[:, 0:SPL], in_=kt_d.ap()[1, :, 0:SPL])
                    nc.gpsimd.dma_start(out=qt[:, 0:SPL], in_=qt_d.ap()[1, :, 0:SPL])
                nc.gpsimd.dma_start(out=kt[:, SPL:T], in_=kt_d.ap()[pair, :, SPL:T])
                nc.gpsimd.dma_start(out=qt[:, SPL:T], in_=qt_d.ap()[pair, :, SPL:T])
                kt_pair[pair], qt_pair[pair] = kt, qt
                for h in (hA, hB):
                    vt_h = singles.tile([128, NB, VW], BF16, tag=f"vt{h}")
                    nc.gpsimd.dma_start(out=vt_h[:, :, :], in_=vt_d.ap()[h])
                    vt[h] = vt_h
                    pgq_h = singles.tile([128, NB], BF16, tag=f"pgq{h}")
                    nc.gpsimd.dma_start(out=pgq_h[:, :], in_=pgq_d.ap()[h])
                    pgq[h] = pgq_h
                    v0g_h = singles.tile([128, 4, D], BF16, tag=f"v0g{h}")
                    nc.gpsimd.dma_start(out=v0g_h[:, :, :], in_=v0g_d.ap()[h])
                    v0g[h] = v0g_h

            outstage = []
            for h in range(HPC):
                out_h = outp.tile([128, NB, D], F32, tag=f"out{h}")
                outstage.append(out_h)

            # Compute, software-pipelined across a flat (pair, window) job
            # list: at step `it` we emit scores+exp for job it, PV for job
            # it-1, normalize+store for it-2.
            jobs = [(pair, w) for pair in range(2) for w in range(NWIN)]
            state = {}
            for it in range(len(jobs) + 2):
                if it < len(jobs):
                    pair, w = jobs[it]
                    qt, kt = qt_pair[pair], kt_pair[pair]
                    pieces = _window_pieces(w)
                    offs = _pack_offsets([p[3] for p in pieces])
                    tot = sum(p[3] for p in pieces)
                    sc = spsum.tile([128, 2 * HB], F32, tag="sc")
                    # Head A's pieces (array rows 0-63, sc cols [0:HB)) are
                    # emitted first and exp'd immediately; then head B's
                    # (rows 64-127, cols [HB:2HB)).  The A and B chains
                    # overlap in the PE array (different row groups), and
                    # the split exp means the next job's A pieces only wait
                    # for exp_A of this job, halving the QK<->exp stall.
                    P = pp.tile([128, 2 * HB], BF16, tag="p")
                    for hi, dlo in ((0, 0), (1, 64)):
                        for (j, qlo, qhi, n), off in zip(pieces, offs):
                            nc.tensor.matmul(
                                out=sc[:, hi * HB + off : hi * HB + off + n],
                                lhsT=kt[dlo : dlo + 64, j * B : (j + 1) * B],
                                rhs=qt[dlo : dlo + 64, qlo * B : (qhi + 1) * B],
                                start=True,
                                stop=True,
                            )
                        nc.scalar.activation(
                            out=P[:, hi * HB : hi * HB + tot],
                            in_=sc[:, hi * HB : hi * HB + tot],
                            func=mybir.ActivationFunctionType.Exp,
                            scale=SCALE,
                        )
                    state[it] = (pair, w, pieces, offs, P)
                if 0 <= it - 1 < len(jobs):
                    pair, w, pieces, offs, P = state[it - 1]
                    ctxs = []
                    for hi, h in ((0, 2 * pair), (1, 2 * pair + 1)):
                        ctx = cpsum.tile([128, 4, VW], F32, tag="ctx")
                        for c in range(4):
                            cb = 4 * w + c  # absolute q block
                            js = [j for (j, qlo, qhi, n) in pieces if qlo <= cb <= qhi]
                            for ji, j in enumerate(js):
                                (jj, qlo, qhi, n), off = next(
                                    (pc, of) for pc, of in zip(pieces, offs) if pc[0] == j
                                )
                                col = hi * HB + off + (cb - qlo) * B
                                nc.tensor.matmul(
                                    out=ctx[:, c, :],
                                    lhsT=P[:, col : col + B],
                                    rhs=vt[h][:, j, :],
                                    start=(ji == 0),
                                    stop=(ji == len(js) - 1),
                                )
                        ctxs.append(ctx)
                    state[it - 1] = (pair, w, ctxs)
                if 0 <= it - 2 < len(jobs):
                    pair, w, ctxs = state.pop(it - 2)
                    for hi, h in ((0, 2 * pair), (1, 2 * pair + 1)):
                        ctx = ctxs[hi]
                        pslice = pgq[h][:, 4 * w : 4 * w + 4]
                        dn = rtp.tile([128, 4], F32, tag="dn")
                        nc.vector.tensor_tensor(
                            out=dn[:, :],
                            in0=ctx[:, :, D],
                            in1=pslice,
                            op=mybir.AluOpType.add,
                        )
                        rt = rtp.tile([128, 4], F32, tag="rt")
                        nc.vector.reciprocal(out=rt[:, :], in_=dn[:, :])
                        g = gp.tile([128, 4, D], BF16, tag="g")
                        nc.vector.tensor_mul(
                            out=g[:, :, :],
                            in0=v0g[h][:, :, :],
                            in1=pslice.broadcast_to([128, 4, D]),
                        )
                        u = up.tile([128, 4, D], F32, tag="u")
                        nc.vector.tensor_tensor(
                            out=u[:, :, :],
                            in0=ctx[:, :, 0:D],
                            in1=g[:, :, :],
                            op=mybir.AluOpType.add,
                        )
                        nc.vector.tensor_mul(
                            out=outstage[h][:, 4 * w : 4 * w + 4, :],
                            in0=u[:, :, :],
                            in1=rt[:, :].broadcast_to([128, 4, D]),
                        )
                        if w == NWIN // 2 - 1:
                            nc.sync.dma_start(
                                out=o_d.ap()[h, :, 0 : NB // 2],
                                in_=outstage[h][:, 0 : NB // 2, :],
                            )
                        elif w == NWIN - 1:
                            nc.sync.dma_start(
                                out=o_d.ap()[h, :, NB // 2 : NB],
                                in_=outstage[h][:, NB // 2 : NB, :],
                            )

    nc.compile()
    _NC_CACHE["nc"] = nc
    return nc


def _host_globals(query, key, value):
    """Host-side tiny pieces: pg = exp(scale * K0 . Q) (zeroed for the first
    two query blocks), and o0 = full-sequence attention output for query 0
    (token 0 masked out, as the reference does via attention_mask[..., 0])."""
    q = np.asarray(query, np.float32)
    k = np.asarray(key, np.float32)
    v = np.asarray(value, np.float32)
    k0 = k[:, :, 0, :]  # (n, h, d)
    sg = np.einsum("nhd,nhtd->nht", k0, q) * SCALE
    pg = np.exp(sg)
    pg[:, :, : 2 * B] = 0.0

    q0 = q[:, :, 0, :]  # (n, h, d)
    s0 = np.einsum("nhd,nhtd->nht", q0, k) * SCALE
    s0[:, :, 0] = -np.inf
    s0 -= s0.max(axis=-1, keepdims=True)
    p0 = np.exp(s0)
    p0 /= p0.sum(axis=-1, keepdims=True)
    o0 = np.einsum("nht,nhtd->nhd", p0, v)
    return pg, o0


def kernel(query_layer, key_layer, value_layer, attention_mask):
    from concourse.bass_utils import run_bass_kernel_spmd

    n, h, t, d = query_layer.shape
    assert (n, h, t, d) == (N_, H, T, D)

    q = np.asarray(query_layer, np.float32)
    k = np.asarray(key_layer, np.float32)
    v = np.asarray(value_layer, np.float32)
    pg, o0 = _host_globals(q, k, v)

    bf16 = ml_dtypes.bfloat16
    qf = q.reshape(n * h, T, D)
    kf = k.reshape(n * h, T, D)
    vf = v.reshape(n * h, T, D)

    # qt/kt: per pair of heads, (128, T) bf16 = [headA dT; headB dT]
    qt_all = np.ascontiguousarray(
        qf.astype(bf16).transpose(0, 2, 1).reshape(n * h // 2, 128, T)
    )
    kt_all = np.ascontiguousarray(
        kf.astype(bf16).transpose(0, 2, 1).reshape(n * h // 2, 128, T)
    )
    # vt: (head, 128, NB, 65): [..., 0:64]=V, [..., 64]=ones
    vt_all = np.empty((n * h, 128, NB, VW), bf16)
    vt_all[:, :, :, 0:D] = vf.reshape(n * h, NB, B, D).transpose(0, 2, 1, 3)
    vt_all[:, :, :, D] = np.ones((), bf16)
    # pgq: (head, 128, NB) = pg in q-partition layout
    pgq_all = np.ascontiguousarray(
        pg.reshape(n * h, NB, B).transpose(0, 2, 1).astype(bf16)
    )
    # v0g: (head, 128, 4, 64) = V[0] replicated
    v0g_all = np.ascontiguousarray(
        np.broadcast_to(
            vf[:, 0, :].astype(bf16)[:, None, None, :], (n * h, 128, 4, D)
        )
    )

    in_maps = []
    for c in range(NCORES):
        s = slice(HPC * c, HPC * (c + 1))
        sp = slice(HPC // 2 * c, HPC // 2 * (c + 1))
        in_maps.append(
            {
                "qt": np.ascontiguousarray(qt_all[sp]),
                "kt": np.ascontiguousarray(kt_all[sp]),
                "vt": np.ascontiguousarray(vt_all[s]),
                "pgq": np.ascontiguousarray(pgq_all[s]),
                "v0g": np.ascontiguousarray(v0g_all[s]),
            }
        )

    nc = _build_nc()
    res = run_bass_kernel_spmd(nc, in_maps, core_ids=list(range(NCORES)))
    _NC_CACHE["last_result"] = res
    out = np.concatenate([r["o"] for r in res.results], axis=0)
    out = out.reshape(n * h, 128, NB, D).transpose(0, 2, 1, 3).reshape(n, h, T, D)
    out = np.ascontiguousarray(out)
    out[:, :, 0, :] = o0
    return out


# revision 20
# speedup vs baseline: 1.3090x; 1.3090x over previous
"""Block-local self-attention (BLOCK=128, 3-block sliding window + global token 0)
for Trainium2, sharded over 8 NeuronCores by (batch*head).

Full shapes: q/k/v (2, 16, 4096, 64) fp32, mask (2, 1, 1, 4096) fp32 (zeros).
Core c handles 4 consecutive (n*16+h) heads, as 2 "head pairs".

Host prepares compute-ready, DMA-friendly layouts (big contiguous descriptors):
  - qt/kt: (pair, 128, T) bf16: rows 0-63 = head A's d, 64-127 = head B's d.
  - vt: (head, 128, NB, 65) bf16: partition = token%128, free = (block, d);
    col 64 = ones (softmax denominator trick).
  - pgq: (head, 128, NB) bf16: exp(scale*K0.Q) in q-partition layout,
    zeroed for query blocks 0,1 (global-token probability, host-computed).
  - v0g: (head, 128, 4, 64) bf16: V[token 0] replicated across partitions
    and the 4 query blocks of a window (for the DVE outer-product add).
  - o: (head, 128, NB, D) fp32 staging layout; host un-permutes after.

Device kernel per (pair, window of 512 queries), software-pipelined:
  - scores for BOTH heads in one (128, 3072) PSUM tile, S^T (key-partition)
    layout: per key block j, two row-tiled matmuls (head A on array rows
    0-63, head B on rows 64-127) run concurrently in the PE array.
  - one exp on ScalarE (scale folded) -> P^T bf16 (128, 3072).
  - PV in q-partition layout: for each (query block c, key block j),
    matmul(lhsT=P_j[:, c cols], rhs=vt_j) accumulates ctx (128, 4, 65)
    PSUM; col 64 = denominator via the ones column. N=65 per matmul, so
    PV streams 780 cols/head/window instead of 1536.
  - normalize on DVE directly from PSUM: denom += pgq, reciprocal,
    ctx += pgq (x) V0 (global-token term), multiply -> fp32 out staging.
Query token 0 (attends the full sequence) is host-computed and patched in.
"""

import math

import numpy as np
import ml_dtypes

N_, H, T, D = 2, 16, 4096, 64
B = 128
NB = T // B            # 32 key/query blocks
HPC = 4                # heads per core
NCORES = 8
WQ = 512               # queries per window
NWIN = T // WQ         # 8 windows per head
SCALE = 1.0 / math.sqrt(D)
BANK = 512             # fp32 elements per PSUM bank (per partition)
VW = D + 1             # vt free width: 64 d + 1 ones


def _window_pieces(w):
    """Pieces for window w: (j, qb_lo, qb_hi, N) with q blocks absolute."""
    qb0, qb1 = 4 * w, 4 * w + 3
    out = []
    for j in range(max(0, qb0 - 1), min(NB - 1, qb1 + 1) + 1):
        qlo = max(qb0, j - 1)
        qhi = min(qb1, j + 1)
        out.append((j, qlo, qhi, (qhi - qlo + 1) * B))
    return out


def _pack_offsets(sizes):
    """Pack piece sizes contiguously from 0 s.t. no piece crosses a 512-elem
    PSUM bank boundary. Returns list of offsets (same order as sizes)."""
    import itertools

    n = len(sizes)
    for perm in itertools.permutations(range(n)):
        off = 0
        offs = [0] * n
        ok = True
        for i in perm:
            sz = sizes[i]
            if off // BANK != (off + sz - 1) // BANK:
                ok = False
                break
            offs[i] = off
            off += sz
        if ok:
            return offs
    raise ValueError(f"cannot pack {sizes}")


_NC_CACHE = {}


def _build_nc():
    if "nc" in _NC_CACHE:
        return _NC_CACHE["nc"]

    import concourse.bacc as bacc
    import concourse.mybir as mybir
    import concourse.tile as tile

    dt = mybir.dt
    F32, BF16 = dt.float32, dt.bfloat16
    HB = 3 * BANK  # per-head columns in the scores tile

    nc = bacc.Bacc("TRN2", target_bir_lowering=False, debug=False)
    qt_d = nc.dram_tensor("qt", [2, 128, T], BF16, kind="ExternalInput")
    kt_d = nc.dram_tensor("kt", [2, 128, T], BF16, kind="ExternalInput")
    vt_d = nc.dram_tensor("vt", [HPC, 128, NB, VW], BF16, kind="ExternalInput")
    pgq_d = nc.dram_tensor("pgq", [HPC, 128, NB], BF16, kind="ExternalInput")
    v0g_d = nc.dram_tensor("v0g", [HPC, 128, 4, D], BF16, kind="ExternalInput")
    o_d = nc.dram_tensor("o", [HPC, 128, NB, D], F32, kind="ExternalOutput")

    with tile.TileContext(nc) as tc:
        with (
            tc.tile_pool(name="singles", bufs=1) as singles,
            tc.tile_pool(name="pp", bufs=2) as pp,
            tc.tile_pool(name="gp", bufs=2) as gp,
            tc.tile_pool(name="up", bufs=2) as up,
            tc.tile_pool(name="rtp", bufs=2) as rtp,
            tc.tile_pool(name="outp", bufs=1) as outp,
            tc.tile_pool(name="spsum", bufs=1, space="PSUM") as spsum,
            tc.tile_pool(name="cpsum", bufs=2, space="PSUM") as cpsum,
        ):
            # Warm the ScalarE exp table during the DMA ramp.
            warm_in = singles.tile([1, 8], F32, tag="warm_in")
            nc.vector.memset(warm_in[:, :], 0.0)
            warm_out = singles.tile([1, 8], BF16, tag="warm_out")
            nc.scalar.activation(
                out=warm_out[:, :],
                in_=warm_in[:, :],
                func=mybir.ActivationFunctionType.Exp,
            )

            # Input loads: plain SWDGE (gpsimd) big contiguous transfers,
            # ordered so pair-0 compute starts ASAP. qt/kt split in two
            # chunks so the first window's blocks arrive early.
            qt_pair, kt_pair = [None] * 2, [None] * 2
            vt, pgq, v0g = [None] * HPC, [None] * HPC, [None] * HPC
            SPL = 6 * B  # first chunk: blocks 0-5 (covers window 0)
            for pair in range(2):
                hA, hB = 2 * pair, 2 * pair + 1
                kt = singles.tile([128, T], BF16, tag=f"kt{pair}")
                qt = singles.tile([128, T], BF16, tag=f"qt{pair}")
                if pair == 0:
                    nc.sync.dma_start(out=kt[:, 0:SPL], in_=kt_d.ap()[0, :, 0:SPL])
                    nc.sync.dma_start(out=qt[:, 0:SPL], in_=qt_d.ap()[0, :, 0:SPL])
                else:
                    nc.gpsimd.dma_start(out=kt[:, 0:SPL], in_=kt_d.ap()[1, :, 0:SPL])
                    nc.gpsimd.dma_start(out=qt[:, 0:SPL], in_=qt_d.ap()[1, :, 0:SPL])
                nc.gpsimd.dma_start(out=kt[:, SPL:T], in_=kt_d.ap()[pair, :, SPL:T])
                nc.gpsimd.dma_start(out=qt[:, SPL:T], in_=qt_d.ap()[pair, :, SPL:T])
                kt_pair[pair], qt_pair[pair] = kt, qt
                for h in (hA, hB):
                    vt_h = singles.tile([128, NB, VW], BF16, tag=f"vt{h}")
                    nc.gpsimd.dma_start(out=vt_h[:, :, :], in_=vt_d.ap()[h])
                    vt[h] = vt_h
                    pgq_h = singles.tile([128, NB], BF16, tag=f"pgq{h}")
                    nc.gpsimd.dma_start(out=pgq_h[:, :], in_=pgq_d.ap()[h])
                    pgq[h] = pgq_h
                    v0g_h = singles.tile([128, 4, D], BF16, tag=f"v0g{h}")
                    nc.gpsimd.dma_start(out=v0g_h[:, :, :], in_=v0g_d.ap()[h])
                    v0g[h] = v0g_h

            outstage = []
            for h in range(HPC):
                out_h = outp.tile([128, NB, D], F32, tag=f"out{h}")
                outstage.append(out_h)

            # Compute, software-pipelined across a flat (pair, window) job
            # list: at step `it` we emit scores+exp for job it, PV for job
            # it-1, normalize+store for it-2.
            jobs = [(pair, w) for pair in range(2) for w in range(NWIN)]
            state = {}
            for it in range(len(jobs) + 2):
                if it < len(jobs):
                    pair, w = jobs[it]
                    qt, kt = qt_pair[pair], kt_pair[pair]
                    pieces = _window_pieces(w)
                    offs = _pack_offsets([p[3] for p in pieces])
                    tot = sum(p[3] for p in pieces)
                    sc = spsum.tile([128, 2 * HB], F32, tag="sc")
                    # Row-tiled pairs: head A on array rows 0-63 writes
                    # cols [0:HB), head B on rows 64-127 writes [HB:2HB).
                    # Adjacent emission lets the PE run them concurrently.
                    for (j, qlo, qhi, n), off in zip(pieces, offs):
                        for hi, dlo in ((0, 0), (1, 64)):
                            nc.tensor.matmul(
                                out=sc[:, hi * HB + off : hi * HB + off + n],
                                lhsT=kt[dlo : dlo + 64, j * B : (j + 1) * B],
                                rhs=qt[dlo : dlo + 64, qlo * B : (qhi + 1) * B],
                                start=True,
                                stop=True,
                            )
                    P = pp.tile([128, 2 * HB], BF16, tag="p")
                    nc.scalar.activation(
                        out=P[:, 0 : HB + tot],
                        in_=sc[:, 0 : HB + tot],
                        func=mybir.ActivationFunctionType.Exp,
                        scale=SCALE,
                    )
                    state[it] = (pair, w, pieces, offs, P)
                if 0 <= it - 1 < len(jobs):
                    pair, w, pieces, offs, P = state[it - 1]
                    ctxs = []
                    for hi, h in ((0, 2 * pair), (1, 2 * pair + 1)):
                        ctx = cpsum.tile([128, 4, VW], F32, tag="ctx")
                        for c in range(4):
                            cb = 4 * w + c  # absolute q block
                            js = [j for (j, qlo, qhi, n) in pieces if qlo <= cb <= qhi]
                            for ji, j in enumerate(js):
                                (jj, qlo, qhi, n), off = next(
                                    (pc, of) for pc, of in zip(pieces, offs) if pc[0] == j
                                )
                                col = hi * HB + off + (cb - qlo) * B
                                nc.tensor.matmul(
                                    out=ctx[:, c, :],
                                    lhsT=P[:, col : col + B],
                                    rhs=vt[h][:, j, :],
                                    start=(ji == 0),
                                    stop=(ji == len(js) - 1),
                                )
                        ctxs.append(ctx)
                    state[it - 1] = (pair, w, ctxs)
                if 0 <= it - 2 < len(jobs):
                    pair, w, ctxs = state.pop(it - 2)
                    for hi, h in ((0, 2 * pair), (1, 2 * pair + 1)):
                        ctx = ctxs[hi]
                        pslice = pgq[h][:, 4 * w : 4 * w + 4]
                        dn = rtp.tile([128, 4], F32, tag="dn")
                        nc.vector.tensor_tensor(
                            out=dn[:, :],
                            in0=ctx[:, :, D],
                            in1=pslice,
                            op=mybir.AluOpType.add,
                        )
                        rt = rtp.tile([128, 4], F32, tag="rt")
                        nc.vector.reciprocal(out=rt[:, :], in_=dn[:, :])
                        g = gp.tile([128, 4, D], BF16, tag="g")
                        nc.vector.tensor_mul(
                            out=g[:, :, :],
                            in0=v0g[h][:, :, :],
                            in1=pslice.broadcast_to([128, 4, D]),
                        )
                        u = up.tile([128, 4, D], F32, tag="u")
                        nc.vector.tensor_tensor(
                            out=u[:, :, :],
                            in0=ctx[:, :, 0:D],
                            in1=g[:, :, :],
                            op=mybir.AluOpType.add,
                        )
                        nc.vector.tensor_mul(
                            out=outstage[h][:, 4 * w : 4 * w + 4, :],
                            in0=u[:, :, :],
                            in1=rt[:, :].broadcast_to([128, 4, D]),
                        )
                        if w % 2 == 1:
                            b0, b1 = (w - 1) * 4, (w + 1) * 4
                            nc.sync.dma_start(
                                out=o_d.ap()[h, :, b0:b1],
                                in_=outstage[h][:, b0:b1, :],
                            )

    nc.compile()
    _NC_CACHE["nc"] = nc
    return nc


def _host_globals(query, key, value):
    """Host-side tiny pieces: pg = exp(scale * K0 . Q) (zeroed for the first
    two query blocks), and o0 = full-sequence attention output for query 0
    (token 0 masked out, as the reference does via attention_mask[..., 0])."""
    q = np.asarray(query, np.float32)
    k = np.asarray(key, np.float32)
    v = np.asarray(value, np.float32)
    k0 = k[:, :, 0, :]  # (n, h, d)
    sg = np.einsum("nhd,nhtd->nht", k0, q) * SCALE
    pg = np.exp(sg)
    pg[:, :, : 2 * B] = 0.0

    q0 = q[:, :, 0, :]  # (n, h, d)
    s0 = np.einsum("nhd,nhtd->nht", q0, k) * SCALE
    s0[:, :, 0] = -np.inf
    s0 -= s0.max(axis=-1, keepdims=True)
    p0 = np.exp(s0)
    p0 /= p0.sum(axis=-1, keepdims=True)
    o0 = np.einsum("nht,nhtd->nhd", p0, v)
    return pg, o0


def kernel(query_layer, key_layer, value_layer, attention_mask):
    from concourse.bass_utils import run_bass_kernel_spmd

    n, h, t, d = query_layer.shape
    assert (n, h, t, d) == (N_, H, T, D)

    q = np.asarray(query_layer, np.float32)
    k = np.asarray(key_layer, np.float32)
    v = np.asarray(value_layer, np.float32)
    pg, o0 = _host_globals(q, k, v)

    bf16 = ml_dtypes.bfloat16
    qf = q.reshape(n * h, T, D)
    kf = k.reshape(n * h, T, D)
    vf = v.reshape(n * h, T, D)

    # qt/kt: per pair of heads, (128, T) bf16 = [headA dT; headB dT]
    qt_all = np.ascontiguousarray(
        qf.astype(bf16).transpose(0, 2, 1).reshape(n * h // 2, 128, T)
    )
    kt_all = np.ascontiguousarray(
        kf.astype(bf16).transpose(0, 2, 1).reshape(n * h // 2, 128, T)
    )
    # vt: (head, 128, NB, 65): [..., 0:64]=V, [..., 64]=ones
    vt_all = np.empty((n * h, 128, NB, VW), bf16)
    vt_all[:, :, :, 0:D] = vf.reshape(n * h, NB, B, D).transpose(0, 2, 1, 3)
    vt_all[:, :, :, D] = np.ones((), bf16)
    # pgq: (head, 128, NB) = pg in q-partition layout
    pgq_all = np.ascontiguousarray(
        pg.reshape(n * h, NB, B).transpose(0, 2, 1).astype(bf16)
    )
    # v0g: (head, 128, 4, 64) = V[0] replicated
    v0g_all = np.ascontiguousarray(
        np.broadcast_to(
            vf[:, 0, :].astype(bf16)[:, None, None, :], (n * h, 128, 4, D)
        )
    )

    in_maps = []
    for c in range(NCORES):
        s = slice(HPC * c, HPC * (c + 1))
        sp = slice(HPC // 2 * c, HPC // 2 * (c + 1))
        in_maps.append(
            {
                "qt": np.ascontiguousarray(qt_all[sp]),
                "kt": np.ascontiguousarray(kt_all[sp]),
                "vt": np.ascontiguousarray(vt_all[s]),
                "pgq": np.ascontiguousarray(pgq_all[s]),
                "v0g": np.ascontiguousarray(v0g_all[s]),
            }
        )

    nc = _build_nc()
    res = run_bass_kernel_spmd(nc, in_maps, core_ids=list(range(NCORES)))
    _NC_CACHE["last_result"] = res
    out = np.concatenate([r["o"] for r in res.results], axis=0)
    out = out.reshape(n * h, 128, NB, D).transpose(0, 2, 1, 3).reshape(n, h, T, D)
    out = np.ascontiguousarray(out)
    out[:, :, 0, :] = o0
    return out


# revision 27
# speedup vs baseline: 1.3290x; 1.0153x over previous
"""Block-local self-attention (BLOCK=128, 3-block sliding window + global token 0)
for Trainium2, sharded over 8 NeuronCores by (batch*head).

Full shapes: q/k/v (2, 16, 4096, 64) fp32, mask (2, 1, 1, 4096) fp32 (zeros).
Core c handles 4 consecutive (n*16+h) heads, as 2 "head pairs".

Host prepares compute-ready, DMA-friendly layouts (big contiguous descriptors):
  - qt/kt: (pair, 128, T) bf16: rows 0-63 = head A's d, 64-127 = head B's d.
  - vt: (head, 128, NB, 65) bf16: partition = token%128, free = (block, d);
    col 64 = ones (softmax denominator trick).
  - pgq: (head, 128, NB) bf16: exp(scale*K0.Q) in q-partition layout,
    zeroed for query blocks 0,1 (global-token probability, host-computed).
  - v0g: (head, 128, 4, 64) bf16: V[token 0] replicated across partitions
    and the 4 query blocks of a window (for the DVE outer-product add).
  - o: (head, 128, NB, D) fp32 staging layout; host un-permutes after.

Device kernel per (pair, window of 512 queries), software-pipelined:
  - scores for BOTH heads in one (128, 3072) PSUM tile, S^T (key-partition)
    layout: per key block j, two row-tiled matmuls (head A on array rows
    0-63, head B on rows 64-127) run concurrently in the PE array.
  - one exp on ScalarE (scale folded) -> P^T bf16 (128, 3072).
  - PV in q-partition layout: for each (query block c, key block j),
    matmul(lhsT=P_j[:, c cols], rhs=vt_j) accumulates ctx (128, 4, 65)
    PSUM; col 64 = denominator via the ones column. N=65 per matmul, so
    PV streams 780 cols/head/window instead of 1536.
  - normalize on DVE directly from PSUM: denom += pgq, reciprocal,
    ctx += pgq (x) V0 (global-token term), multiply -> fp32 out staging.
Query token 0 (attends the full sequence) is host-computed and patched in.
"""

import math

import numpy as np
import ml_dtypes

N_, H, T, D = 2, 16, 4096, 64
B = 128
NB = T // B            # 32 key/query blocks
HPC = 4                # heads per core
NCORES = 8
WQ = 512               # queries per window
NWIN = T // WQ         # 8 windows per head
SCALE = 1.0 / math.sqrt(D)
BANK = 512             # fp32 elements per PSUM bank (per partition)
VW = D + 1             # vt free width: 64 d + 1 ones


def _window_pieces(w):
    """Pieces for window w: (j, qb_lo, qb_hi, N) with q blocks absolute."""
    qb0, qb1 = 4 * w, 4 * w + 3
    out = []
    for j in range(max(0, qb0 - 1), min(NB - 1, qb1 + 1) + 1):
        qlo = max(qb0, j - 1)
        qhi = min(qb1, j + 1)
        out.append((j, qlo, qhi, (qhi - qlo + 1) * B))
    return out


def _pack_offsets(sizes):
    """Pack piece sizes contiguously from 0 s.t. no piece crosses a 512-elem
    PSUM bank boundary. Returns list of offsets (same order as sizes)."""
    import itertools

    n = len(sizes)
    for perm in itertools.permutations(range(n)):
        off = 0
        offs = [0] * n
        ok = True
        for i in perm:
            sz = sizes[i]
            if off // BANK != (off + sz - 1) // BANK:
                ok = False
                break
            offs[i] = off
            off += sz
        if ok:
            return offs
    raise ValueError(f"cannot pack {sizes}")


_NC_CACHE = {}


def _build_nc():
    if "nc" in _NC_CACHE:
        return _NC_CACHE["nc"]

    import concourse.bacc as bacc
    import concourse.mybir as mybir
    import concourse.tile as tile

    dt = mybir.dt
    F32, BF16 = dt.float32, dt.bfloat16
    HB = 3 * BANK  # per-head columns in the scores tile

    nc = bacc.Bacc("TRN2", target_bir_lowering=False, debug=False)
    qt_d = nc.dram_tensor("qt", [2, 128, T], BF16, kind="ExternalInput")
    kt_d = nc.dram_tensor("kt", [2, 128, T], BF16, kind="ExternalInput")
    vt_d = nc.dram_tensor("vt", [HPC, 128, NB, VW], BF16, kind="ExternalInput")
    pgq_d = nc.dram_tensor("pgq", [HPC, 128, NB], BF16, kind="ExternalInput")
    v0g_d = nc.dram_tensor("v0g", [HPC, 128, 4, D], BF16, kind="ExternalInput")
    o_d = nc.dram_tensor("o", [HPC, 128, NB, D], F32, kind="ExternalOutput")

    with tile.TileContext(nc) as tc:
        with (
            tc.tile_pool(name="singles", bufs=1) as singles,
            tc.tile_pool(name="pp", bufs=2) as pp,
            tc.tile_pool(name="gp", bufs=2) as gp,
            tc.tile_pool(name="up", bufs=2) as up,
            tc.tile_pool(name="rtp", bufs=2) as rtp,
            tc.tile_pool(name="outp", bufs=1) as outp,
            tc.tile_pool(name="spsum", bufs=1, space="PSUM") as spsum,
            tc.tile_pool(name="cpsum", bufs=2, space="PSUM") as cpsum,
        ):
            # Input loads: plain SWDGE (gpsimd) big contiguous transfers,
            # ordered so pair-0 compute starts ASAP. qt/kt split in two
            # chunks so the first window's blocks arrive early.
            qt_pair, kt_pair = [None] * 2, [None] * 2
            vt, pgq, v0g = [None] * HPC, [None] * HPC, [None] * HPC
            SPL = 6 * B  # first chunk: blocks 0-5 (covers window 0)
            for pair in range(2):
                hA, hB = 2 * pair, 2 * pair + 1
                kt = singles.tile([128, T], BF16, tag=f"kt{pair}")
                qt = singles.tile([128, T], BF16, tag=f"qt{pair}")
                if pair == 0:
                    nc.sync.dma_start(out=kt[:, 0:SPL], in_=kt_d.ap()[0, :, 0:SPL])
                    nc.scalar.dma_start(out=qt[:, 0:SPL], in_=qt_d.ap()[0, :, 0:SPL])
                else:
                    nc.gpsimd.dma_start(out=kt[:, 0:SPL], in_=kt_d.ap()[1, :, 0:SPL])
                    nc.gpsimd.dma_start(out=qt[:, 0:SPL], in_=qt_d.ap()[1, :, 0:SPL])
                nc.gpsimd.dma_start(out=kt[:, SPL:T], in_=kt_d.ap()[pair, :, SPL:T])
                nc.gpsimd.dma_start(out=qt[:, SPL:T], in_=qt_d.ap()[pair, :, SPL:T])
                kt_pair[pair], qt_pair[pair] = kt, qt
                for h in (hA, hB):
                    vt_h = singles.tile([128, NB, VW], BF16, tag=f"vt{h}")
                    nc.gpsimd.dma_start(out=vt_h[:, :, :], in_=vt_d.ap()[h])
                    vt[h] = vt_h
                    pgq_h = singles.tile([128, NB], BF16, tag=f"pgq{h}")
                    nc.gpsimd.dma_start(out=pgq_h[:, :], in_=pgq_d.ap()[h])
                    pgq[h] = pgq_h
                    v0g_h = singles.tile([128, 4, D], BF16, tag=f"v0g{h}")
                    nc.gpsimd.dma_start(out=v0g_h[:, :, :], in_=v0g_d.ap()[h])
                    v0g[h] = v0g_h

            # Warm the ScalarE exp table during the DMA ramp.
            warm_in = singles.tile([1, 8], F32, tag="warm_in")
            nc.vector.memset(warm_in[:, :], 0.0)
            warm_out = singles.tile([1, 8], BF16, tag="warm_out")
            nc.scalar.activation(
                out=warm_out[:, :],
                in_=warm_in[:, :],
                func=mybir.ActivationFunctionType.Exp,
            )

            outstage = []
            for h in range(HPC):
                out_h = outp.tile([128, NB, D], F32, tag=f"out{h}")
                outstage.append(out_h)

            # Compute, software-pipelined across a flat (pair, window) job
            # list: at step `it` we emit scores+exp for job it, PV for job
            # it-1, normalize+store for it-2.
            jobs = [(pair, w) for pair in range(2) for w in range(NWIN)]
            state = {}
            for it in range(len(jobs) + 2):
                if it < len(jobs):
                    pair, w = jobs[it]
                    qt, kt = qt_pair[pair], kt_pair[pair]
                    pieces = _window_pieces(w)
                    offs = _pack_offsets([p[3] for p in pieces])
                    tot = sum(p[3] for p in pieces)
                    sc = spsum.tile([128, 2 * HB], F32, tag="sc")
                    # Row-tiled pairs: head A on array rows 0-63 writes
                    # cols [0:HB), head B on rows 64-127 writes [HB:2HB).
                    # Adjacent emission lets the PE run them concurrently.
                    for (j, qlo, qhi, n), off in zip(pieces, offs):
                        for hi, dlo in ((0, 0), (1, 64)):
                            nc.tensor.matmul(
                                out=sc[:, hi * HB + off : hi * HB + off + n],
                                lhsT=kt[dlo : dlo + 64, j * B : (j + 1) * B],
                                rhs=qt[dlo : dlo + 64, qlo * B : (qhi + 1) * B],
                                start=True,
                                stop=True,
                            )
                    P = pp.tile([128, 2 * HB], BF16, tag="p")
                    nc.scalar.activation(
                        out=P[:, 0 : HB + tot],
                        in_=sc[:, 0 : HB + tot],
                        func=mybir.ActivationFunctionType.Exp,
                        scale=SCALE,
                    )
                    state[it] = (pair, w, pieces, offs, P)
                if 0 <= it - 1 < len(jobs):
                    pair, w, pieces, offs, P = state[it - 1]
                    ctxs = []
                    for hi, h in ((0, 2 * pair), (1, 2 * pair + 1)):
                        ctx = cpsum.tile([128, 4, VW], F32, tag="ctx")
                        for c in range(4):
                            cb = 4 * w + c  # absolute q block
                            js = [j for (j, qlo, qhi, n) in pieces if qlo <= cb <= qhi]
                            for ji, j in enumerate(js):
                                (jj, qlo, qhi, n), off = next(
                                    (pc, of) for pc, of in zip(pieces, offs) if pc[0] == j
                                )
                                col = hi * HB + off + (cb - qlo) * B
                                nc.tensor.matmul(
                                    out=ctx[:, c, :],
                                    lhsT=P[:, col : col + B],
                                    rhs=vt[h][:, j, :],
                                    start=(ji == 0),
                                    stop=(ji == len(js) - 1),
                                )
                        ctxs.append(ctx)
                    state[it - 1] = (pair, w, ctxs)
                if 0 <= it - 2 < len(jobs):
                    pair, w, ctxs = state.pop(it - 2)
                    for hi, h in ((0, 2 * pair), (1, 2 * pair + 1)):
                        ctx = ctxs[hi]
                        pslice = pgq[h][:, 4 * w : 4 * w + 4]
                        dn = rtp.tile([128, 4], F32, tag="dn")
                        nc.vector.tensor_tensor(
                            out=dn[:, :],
                            in0=ctx[:, :, D],
                            in1=pslice,
                            op=mybir.AluOpType.add,
                        )
                        rt = rtp.tile([128, 4], F32, tag="rt")
                        nc.vector.reciprocal(out=rt[:, :], in_=dn[:, :])
                        g = gp.tile([128, 4, D], BF16, tag="g")
                        nc.vector.tensor_mul(
                            out=g[:, :, :],
                            in0=v0g[h][:, :, :],
                            in1=pslice.broadcast_to([128, 4, D]),
                        )
                        u = up.tile([128, 4, D], F32, tag="u")
                        nc.vector.tensor_tensor(
                            out=u[:, :, :],
                            in0=ctx[:, :, 0:D],
                            in1=g[:, :, :],
                            op=mybir.AluOpType.add,
                        )
                        nc.vector.tensor_mul(
                            out=outstage[h][:, 4 * w : 4 * w + 4, :],
                            in0=u[:, :, :],
                            in1=rt[:, :].broadcast_to([128, 4, D]),
                        )
                        if w % 2 == 1:
                            b0, b1 = (w - 1) * 4, (w + 1) * 4
                            nc.sync.dma_start(
                                out=o_d.ap()[h, :, b0:b1],
                                in_=outstage[h][:, b0:b1, :],
                            )

    nc.compile()
    _NC_CACHE["nc"] = nc
    return nc


def _host_globals(query, key, value):
    """Host-side tiny pieces: pg = exp(scale * K0 . Q) (zeroed for the first
    two query blocks), and o0 = full-sequence attention output for query 0
    (token 0 masked out, as the reference does via attention_mask[..., 0])."""
    q = np.asarray(query, np.float32)
    k = np.asarray(key, np.float32)
    v = np.asarray(value, np.float32)
    k0 = k[:, :, 0, :]  # (n, h, d)
    sg = np.einsum("nhd,nhtd->nht", k0, q) * SCALE
    pg = np.exp(sg)
    pg[:, :, : 2 * B] = 0.0

    q0 = q[:, :, 0, :]  # (n, h, d)
    s0 = np.einsum("nhd,nhtd->nht", q0, k) * SCALE
    s0[:, :, 0] = -np.inf
    s0 -= s0.max(axis=-1, keepdims=True)
    p0 = np.exp(s0)
    p0 /= p0.sum(axis=-1, keepdims=True)
    o0 = np.einsum("nht,nhtd->nhd", p0, v)
    return pg, o0


def kernel(query_layer, key_layer, value_layer, attention_mask):
    from concourse.bass_utils import run_bass_kernel_spmd

    n, h, t, d = query_layer.shape
    assert (n, h, t, d) == (N_, H, T, D)

    q = np.asarray(query_layer, np.float32)
    k = np.asarray(key_layer, np.float32)
    v = np.asarray(value_layer, np.float32)
    pg, o0 = _host_globals(q, k, v)

    bf16 = ml_dtypes.bfloat16
    qf = q.reshape(n * h, T, D)
    kf = k.reshape(n * h, T, D)
    vf = v.reshape(n * h, T, D)

    # qt/kt: per pair of heads, (128, T) bf16 = [headA dT; headB dT]
    qt_all = np.ascontiguousarray(
        qf.astype(bf16).transpose(0, 2, 1).reshape(n * h // 2, 128, T)
    )
    kt_all = np.ascontiguousarray(
        kf.astype(bf16).transpose(0, 2, 1).reshape(n * h // 2, 128, T)
    )
    # vt: (head, 128, NB, 65): [..., 0:64]=V, [..., 64]=ones
    vt_all = np.empty((n * h, 128, NB, VW), bf16)
    vt_all[:, :, :, 0:D] = vf.reshape(n * h, NB, B, D).transpose(0, 2, 1, 3)
    vt_all[:, :, :, D] = np.ones((), bf16)
    # pgq: (head, 128, NB) = pg in q-partition layout
    pgq_all = np.ascontiguousarray(
        pg.reshape(n * h, NB, B).transpose(0, 2, 1).astype(bf16)
    )
    # v0g: (head, 128, 4, 64) = V[0] replicated
    v0g_all = np.ascontiguousarray(
        np.broadcast_to(
            vf[:, 0, :].astype(bf16)[:, None, None, :], (n * h, 128, 4, D)
        )
    )

    in_maps = []
    for c in range(NCORES):
        s = slice(HPC * c, HPC * (c + 1))
        sp = slice(HPC // 2 * c, HPC // 2 * (c + 1))
        in_maps.append(
            {
                "qt": np.ascontiguousarray(qt_all[sp]),
                "kt": np.ascontiguousarray(kt_all[sp]),
                "vt": np.ascontiguousarray(vt_all[s]),
                "pgq": np.ascontiguousarray(pgq_all[s]),
                "v0g": np.ascontiguousarray(v0g_all[s]),
            }
        )

    nc = _build_nc()
    res = run_bass_kernel_spmd(nc, in_maps, core_ids=list(range(NCORES)))
    _NC_CACHE["last_result"] = res
    out = np.concatenate([r["o"] for r in res.results], axis=0)
    out = out.reshape(n * h, 128, NB, D).transpose(0, 2, 1, 3).reshape(n, h, T, D)
    out = np.ascontiguousarray(out)
    out[:, :, 0, :] = o0
    return out


# revision 29
# speedup vs baseline: 1.3363x; 1.0055x over previous
"""Block-local self-attention (BLOCK=128, 3-block sliding window + global token 0)
for Trainium2, sharded over 8 NeuronCores by (batch*head).

Full shapes: q/k/v (2, 16, 4096, 64) fp32, mask (2, 1, 1, 4096) fp32 (zeros).
Core c handles 4 consecutive (n*16+h) heads, as 2 "head pairs".

Host prepares compute-ready, DMA-friendly layouts (big contiguous descriptors):
  - qt/kt: (pair, 128, T) bf16: rows 0-63 = head A's d, 64-127 = head B's d.
  - vt: (head, 128, NB, 65) bf16: partition = token%128, free = (block, d);
    col 64 = ones (softmax denominator trick).
  - pgq: (head, 128, NB) bf16: exp(scale*K0.Q) in q-partition layout,
    zeroed for query blocks 0,1 (global-token probability, host-computed).
  - v0g: (head, 128, 4, 64) bf16: V[token 0] replicated across partitions
    and the 4 query blocks of a window (for the DVE outer-product add).
  - o: (head, 128, NB, D) fp32 staging layout; host un-permutes after.

Device kernel per (pair, window of 512 queries), software-pipelined:
  - scores for BOTH heads in one (128, 3072) PSUM tile, S^T (key-partition)
    layout: per key block j, two row-tiled matmuls (head A on array rows
    0-63, head B on rows 64-127) run concurrently in the PE array.
  - one exp on ScalarE (scale folded) -> P^T bf16 (128, 3072).
  - PV in q-partition layout: for each (query block c, key block j),
    matmul(lhsT=P_j[:, c cols], rhs=vt_j) accumulates ctx (128, 4, 65)
    PSUM; col 64 = denominator via the ones column. N=65 per matmul, so
    PV streams 780 cols/head/window instead of 1536.
  - normalize on DVE directly from PSUM: denom += pgq, reciprocal,
    ctx += pgq (x) V0 (global-token term), multiply -> fp32 out staging.
Query token 0 (attends the full sequence) is host-computed and patched in.
"""

import math

import numpy as np
import ml_dtypes

N_, H, T, D = 2, 16, 4096, 64
B = 128
NB = T // B            # 32 key/query blocks
HPC = 4                # heads per core
NCORES = 8
WQ = 512               # queries per window
NWIN = T // WQ         # 8 windows per head
SCALE = 1.0 / math.sqrt(D)
BANK = 512             # fp32 elements per PSUM bank (per partition)
VW = D + 1             # vt free width: 64 d + 1 ones


def _window_pieces(w):
    """Pieces for window w: (j, qb_lo, qb_hi, N) with q blocks absolute."""
    qb0, qb1 = 4 * w, 4 * w + 3
    out = []
    for j in range(max(0, qb0 - 1), min(NB - 1, qb1 + 1) + 1):
        qlo = max(qb0, j - 1)
        qhi = min(qb1, j + 1)
        out.append((j, qlo, qhi, (qhi - qlo + 1) * B))
    return out


def _pack_offsets(sizes):
    """Pack piece sizes contiguously from 0 s.t. no piece crosses a 512-elem
    PSUM bank boundary. Returns list of offsets (same order as sizes)."""
    import itertools

    n = len(sizes)
    for perm in itertools.permutations(range(n)):
        off = 0
        offs = [0] * n
        ok = True
        for i in perm:
            sz = sizes[i]
            if off // BANK != (off + sz - 1) // BANK:
                ok = False
                break
            offs[i] = off
            off += sz
        if ok:
            return offs
    raise ValueError(f"cannot pack {sizes}")


_NC_CACHE = {}


def _build_nc():
    if "nc" in _NC_CACHE:
        return _NC_CACHE["nc"]

    import concourse.bacc as bacc
    import concourse.mybir as mybir
    import concourse.tile as tile

    dt = mybir.dt
    F32, BF16 = dt.float32, dt.bfloat16
    HB = 3 * BANK  # per-head columns in the scores tile

    nc = bacc.Bacc("TRN2", target_bir_lowering=False, debug=False)
    qt_d = nc.dram_tensor("qt", [2, 128, T], BF16, kind="ExternalInput")
    kt_d = nc.dram_tensor("kt", [2, 128, T], BF16, kind="ExternalInput")
    vt_d = nc.dram_tensor("vt", [HPC, 128, NB, VW], BF16, kind="ExternalInput")
    pgq_d = nc.dram_tensor("pgq", [HPC, 128, NB], BF16, kind="ExternalInput")
    v0g_d = nc.dram_tensor("v0g", [HPC, 128, 4, D], BF16, kind="ExternalInput")
    o_d = nc.dram_tensor("o", [HPC, 128, NB, D], F32, kind="ExternalOutput")

    with tile.TileContext(nc) as tc:
        with (
            tc.tile_pool(name="singles", bufs=1) as singles,
            tc.tile_pool(name="pp", bufs=2) as pp,
            tc.tile_pool(name="gp", bufs=2) as gp,
            tc.tile_pool(name="up", bufs=2) as up,
            tc.tile_pool(name="rtp", bufs=2) as rtp,
            tc.tile_pool(name="outp", bufs=1) as outp,
            tc.tile_pool(name="spsum", bufs=1, space="PSUM") as spsum,
            tc.tile_pool(name="cpsum", bufs=2, space="PSUM") as cpsum,
        ):
            # Input loads: plain SWDGE (gpsimd) big contiguous transfers,
            # ordered so pair-0 compute starts ASAP. qt/kt split in two
            # chunks so the first window's blocks arrive early.
            qt_pair, kt_pair = [None] * 2, [None] * 2
            vt, pgq, v0g = [None] * HPC, [None] * HPC, [None] * HPC
            SPL = 6 * B  # first chunk: blocks 0-5 (covers window 0)
            for pair in range(2):
                hA, hB = 2 * pair, 2 * pair + 1
                kt = singles.tile([128, T], BF16, tag=f"kt{pair}")
                qt = singles.tile([128, T], BF16, tag=f"qt{pair}")
                if pair == 0:
                    nc.sync.dma_start(out=kt[:, 0:SPL], in_=kt_d.ap()[0, :, 0:SPL])
                    nc.scalar.dma_start(out=qt[:, 0:SPL], in_=qt_d.ap()[0, :, 0:SPL])
                else:
                    nc.gpsimd.dma_start(out=kt[:, 0:SPL], in_=kt_d.ap()[1, :, 0:SPL])
                    nc.gpsimd.dma_start(out=qt[:, 0:SPL], in_=qt_d.ap()[1, :, 0:SPL])
                nc.gpsimd.dma_start(out=kt[:, SPL:T], in_=kt_d.ap()[pair, :, SPL:T])
                nc.gpsimd.dma_start(out=qt[:, SPL:T], in_=qt_d.ap()[pair, :, SPL:T])
                kt_pair[pair], qt_pair[pair] = kt, qt
                for h in (hA, hB):
                    vt_h = singles.tile([128, NB, VW], BF16, tag=f"vt{h}")
                    nc.gpsimd.dma_start(out=vt_h[:, :, :], in_=vt_d.ap()[h])
                    vt[h] = vt_h
                    pgq_h = singles.tile([128, NB], BF16, tag=f"pgq{h}")
                    nc.gpsimd.dma_start(out=pgq_h[:, :], in_=pgq_d.ap()[h])
                    pgq[h] = pgq_h
                    v0g_h = singles.tile([128, 4, D], BF16, tag=f"v0g{h}")
                    nc.gpsimd.dma_start(out=v0g_h[:, :, :], in_=v0g_d.ap()[h])
                    v0g[h] = v0g_h

            # Warm the ScalarE exp table during the DMA ramp.
            warm_in = singles.tile([1, 8], F32, tag="warm_in")
            nc.vector.memset(warm_in[:, :], 0.0)
            warm_out = singles.tile([1, 8], BF16, tag="warm_out")
            nc.scalar.activation(
                out=warm_out[:, :],
                in_=warm_in[:, :],
                func=mybir.ActivationFunctionType.Exp,
            )

            # Output staging in 8-block tiles matching store granularity,
            # so a store in flight never blocks the next windows' normalize
            # writes (Tile tracks deps per tile).
            outstage = []
            for h in range(HPC):
                tiles_h = []
                for s in range(4):
                    out_hs = outp.tile([128, 8, D], F32, tag=f"out{h}_{s}")
                    tiles_h.append(out_hs)
                outstage.append(tiles_h)

            # Compute, software-pipelined across a flat (pair, window) job
            # list: at step `it` we emit scores+exp for job it, PV for job
            # it-1, normalize+store for it-2.
            jobs = [(pair, w) for pair in range(2) for w in range(NWIN)]
            state = {}
            for it in range(len(jobs) + 2):
                if it < len(jobs):
                    pair, w = jobs[it]
                    qt, kt = qt_pair[pair], kt_pair[pair]
                    pieces = _window_pieces(w)
                    offs = _pack_offsets([p[3] for p in pieces])
                    tot = sum(p[3] for p in pieces)
                    sc = spsum.tile([128, 2 * HB], F32, tag="sc")
                    # Row-tiled pairs: head A on array rows 0-63 writes
                    # cols [0:HB), head B on rows 64-127 writes [HB:2HB).
                    # Adjacent emission lets the PE run them concurrently.
                    for (j, qlo, qhi, n), off in zip(pieces, offs):
                        for hi, dlo in ((0, 0), (1, 64)):
                            nc.tensor.matmul(
                                out=sc[:, hi * HB + off : hi * HB + off + n],
                                lhsT=kt[dlo : dlo + 64, j * B : (j + 1) * B],
                                rhs=qt[dlo : dlo + 64, qlo * B : (qhi + 1) * B],
                                start=True,
                                stop=True,
                            )
                    P = pp.tile([128, 2 * HB], BF16, tag="p")
                    nc.scalar.activation(
                        out=P[:, 0 : HB + tot],
                        in_=sc[:, 0 : HB + tot],
                        func=mybir.ActivationFunctionType.Exp,
                        scale=SCALE,
                    )
                    state[it] = (pair, w, pieces, offs, P)
                if 0 <= it - 1 < len(jobs):
                    pair, w, pieces, offs, P = state[it - 1]
                    ctxs = []
                    for hi, h in ((0, 2 * pair), (1, 2 * pair + 1)):
                        ctx = cpsum.tile([128, 4, VW], F32, tag="ctx")
                        for c in range(4):
                            cb = 4 * w + c  # absolute q block
                            js = [j for (j, qlo, qhi, n) in pieces if qlo <= cb <= qhi]
                            for ji, j in enumerate(js):
                                (jj, qlo, qhi, n), off = next(
                                    (pc, of) for pc, of in zip(pieces, offs) if pc[0] == j
                                )
                                col = hi * HB + off + (cb - qlo) * B
                                nc.tensor.matmul(
                                    out=ctx[:, c, :],
                                    lhsT=P[:, col : col + B],
                                    rhs=vt[h][:, j, :],
                                    start=(ji == 0),
                                    stop=(ji == len(js) - 1),
                                )
                        ctxs.append(ctx)
                    state[it - 1] = (pair, w, ctxs)
                if 0 <= it - 2 < len(jobs):
                    pair, w, ctxs = state.pop(it - 2)
                    for hi, h in ((0, 2 * pair), (1, 2 * pair + 1)):
                        ctx = ctxs[hi]
                        pslice = pgq[h][:, 4 * w : 4 * w + 4]
                        # g first (SBUF-only), then the two ctx readers
                        # back-to-back so the PSUM bank frees ASAP for the
                        # next job's PV.
                        g = gp.tile([128, 4, D], BF16, tag="g")
                        nc.vector.tensor_mul(
                            out=g[:, :, :],
                            in0=v0g[h][:, :, :],
                            in1=pslice.broadcast_to([128, 4, D]),
                        )
                        dn = rtp.tile([128, 4], F32, tag="dn")
                        nc.vector.tensor_tensor(
                            out=dn[:, :],
                            in0=ctx[:, :, D],
                            in1=pslice,
                            op=mybir.AluOpType.add,
                        )
                        u = up.tile([128, 4, D], F32, tag="u")
                        nc.vector.tensor_tensor(
                            out=u[:, :, :],
                            in0=ctx[:, :, 0:D],
                            in1=g[:, :, :],
                            op=mybir.AluOpType.add,
                        )
                        rt = rtp.tile([128, 4], F32, tag="rt")
                        nc.vector.reciprocal(out=rt[:, :], in_=dn[:, :])
                        ostage = outstage[h][w // 2]
                        nc.vector.tensor_mul(
                            out=ostage[:, (w % 2) * 4 : (w % 2) * 4 + 4, :],
                            in0=u[:, :, :],
                            in1=rt[:, :].broadcast_to([128, 4, D]),
                        )
                        if w % 2 == 1:
                            b0 = (w - 1) * 4
                            nc.sync.dma_start(
                                out=o_d.ap()[h, :, b0 : b0 + 8],
                                in_=ostage[:, :, :],
                            )

    nc.compile()
    _NC_CACHE["nc"] = nc
    return nc


def _host_globals(query, key, value):
    """Host-side tiny pieces: pg = exp(scale * K0 . Q) (zeroed for the first
    two query blocks), and o0 = full-sequence attention output for query 0
    (token 0 masked out, as the reference does via attention_mask[..., 0])."""
    q = np.asarray(query, np.float32)
    k = np.asarray(key, np.float32)
    v = np.asarray(value, np.float32)
    k0 = k[:, :, 0, :]  # (n, h, d)
    sg = np.einsum("nhd,nhtd->nht", k0, q) * SCALE
    pg = np.exp(sg)
    pg[:, :, : 2 * B] = 0.0

    q0 = q[:, :, 0, :]  # (n, h, d)
    s0 = np.einsum("nhd,nhtd->nht", q0, k) * SCALE
    s0[:, :, 0] = -np.inf
    s0 -= s0.max(axis=-1, keepdims=True)
    p0 = np.exp(s0)
    p0 /= p0.sum(axis=-1, keepdims=True)
    o0 = np.einsum("nht,nhtd->nhd", p0, v)
    return pg, o0


def kernel(query_layer, key_layer, value_layer, attention_mask):
    from concourse.bass_utils import run_bass_kernel_spmd

    n, h, t, d = query_layer.shape
    assert (n, h, t, d) == (N_, H, T, D)

    q = np.asarray(query_layer, np.float32)
    k = np.asarray(key_layer, np.float32)
    v = np.asarray(value_layer, np.float32)
    pg, o0 = _host_globals(q, k, v)

    bf16 = ml_dtypes.bfloat16
    qf = q.reshape(n * h, T, D)
    kf = k.reshape(n * h, T, D)
    vf = v.reshape(n * h, T, D)

    # qt/kt: per pair of heads, (128, T) bf16 = [headA dT; headB dT]
    qt_all = np.ascontiguousarray(
        qf.astype(bf16).transpose(0, 2, 1).reshape(n * h // 2, 128, T)
    )
    kt_all = np.ascontiguousarray(
        kf.astype(bf16).transpose(0, 2, 1).reshape(n * h // 2, 128, T)
    )
    # vt: (head, 128, NB, 65): [..., 0:64]=V, [..., 64]=ones
    vt_all = np.empty((n * h, 128, NB, VW), bf16)
    vt_all[:, :, :, 0:D] = vf.reshape(n * h, NB, B, D).transpose(0, 2, 1, 3)
    vt_all[:, :, :, D] = np.ones((), bf16)
    # pgq: (head, 128, NB) = pg in q-partition layout
    pgq_all = np.ascontiguousarray(
        pg.reshape(n * h, NB, B).transpose(0, 2, 1).astype(bf16)
    )
    # v0g: (head, 128, 4, 64) = V[0] replicated
    v0g_all = np.ascontiguousarray(
        np.broadcast_to(
            vf[:, 0, :].astype(bf16)[:, None, None, :], (n * h, 128, 4, D)
        )
    )

    in_maps = []
    for c in range(NCORES):
        s = slice(HPC * c, HPC * (c + 1))
        sp = slice(HPC // 2 * c, HPC // 2 * (c + 1))
        in_maps.append(
            {
                "qt": np.ascontiguousarray(qt_all[sp]),
                "kt": np.ascontiguousarray(kt_all[sp]),
                "vt": np.ascontiguousarray(vt_all[s]),
                "pgq": np.ascontiguousarray(pgq_all[s]),
                "v0g": np.ascontiguousarray(v0g_all[s]),
            }
        )

    nc = _build_nc()
    res = run_bass_kernel_spmd(nc, in_maps, core_ids=list(range(NCORES)))
    _NC_CACHE["last_result"] = res
    out = np.concatenate([r["o"] for r in res.results], axis=0)
    out = out.reshape(n * h, 128, NB, D).transpose(0, 2, 1, 3).reshape(n, h, T, D)
    out = np.ascontiguousarray(out)
    out[:, :, 0, :] = o0
    return out
